# revision 15
# baseline (speedup 1.0000x reference)
"""Trainium2 Bass kernel for nn_DynamicDictionaryLearning (vq_codebook).

Computation (full shapes):
    query_embed = (basic_queries @ W_mlp + b_mlp).reshape(T, R, D)    # (T, R*D)
    dynamic_queries = einsum('btr,trd->btd', query_weights, query_embed)
    basic_expanded  = broadcast(basic_queries, (B, T, D))

Sharding (8 NeuronCores, one chip):
    Stage 1 (token-MLP expansion) is tensor-sharded over the R*D output dim:
    core r computes qe_r = basic_queries @ W_mlp[:, r*D:(r+1)*D] + b_r for
    ALL tokens, reading only 1/8th of W_mlp per core.

    An on-chip AllToAll redistributes qe so core c holds all R slices for
    its 128-token slice.  Stage 2 (weighted sum over R) runs as dense PE
    matmuls with block-diagonal qw tiles: contraction packs (r x 16
    tokens) = 128 (r-major), output packs (8 batch x 16 tokens) = 128
    (b-major), so every DMA walks both sides in matching flat order with
    contiguous lines (the DMA queues are descriptor-bound).

    The pipeline is chunked over NQ D-slices; stage2(j) trails stage1 by
    two chunks so each AllToAll (~25-60us) hides under PE work.  Stage 1
    runs contraction-outermost with 8 concurrent PSUM chains so the PE
    starts as soon as the first (bq, W) tile pair lands.  Large strided
    DMAs are split in half across queues to cut their single-queue
    latency.

    basic_expanded is a pure broadcast of an input -> host-side view.
"""

import os

import numpy as np
import ml_dtypes

import concourse.bass as bass
import concourse.mybir as mybir
import concourse.tile as tile
from concourse import bacc
from concourse.bass_utils import run_bass_kernel_spmd

# Problem shapes (hardcoded per spec)
D = 2048
T = 1024
R = 8
B = 32
NC = 8
TS = T // NC          # 128 tokens per core (stage-2 ownership)
P = 128
KT = D // P           # 16 contraction tiles
MT = T // P           # 8 token tiles (stage 1)
NQ = int(os.environ.get("KNQ", "4"))   # pipeline chunks (D-slices)
NW = D // NQ          # cols per chunk
NB = NW // 512        # PSUM-bank-sized n-splits per chunk
TG = TS // 16         # 8 token groups of 16 (stage 2)
BG = B // 8           # 4 batch groups of 8 (stage 2)

F32 = mybir.dt.float32
F32R = mybir.dt.float32r
BF16 = mybir.dt.bfloat16

# matmul-operand dtype: "bf16" (fast, ~4e-3 rel err) or "f32r"
# (full fp32 data, ~3e-4 rel err, ~2x the DMA bytes)
USE_F32R = os.environ.get("KF32R", "0") == "1"
DT_MM = F32R if USE_F32R else BF16
NP_MM = np.float32 if USE_F32R else ml_dtypes.bfloat16

_cache = {}


def _build_nc():
    nc = bacc.Bacc("TRN2", target_bir_lowering=False, num_devices=NC)

    bqT = nc.dram_tensor("bqT", [D, T], DT_MM, kind="ExternalInput")
    Wc = nc.dram_tensor("Wc", [D, D], DT_MM, kind="ExternalInput")
    biasr = nc.dram_tensor("biasr", [P, D], F32, kind="ExternalInput")
    # block-diagonal qw tiles, packed (128, 32*128) for one big-line DMA
    Lt = nc.dram_tensor("Lt", [P, TG * BG * P], DT_MM, kind="ExternalInput")
    # chunk-major layout: (t, n) runs are contiguous per (j, b)
    dq = nc.dram_tensor("dq", [NQ, B, TS, NW], F32, kind="ExternalOutput")

    bqT_t = bqT.rearrange("(kt p) m -> kt p m", p=P)   # (16, 128, 1024)
    Wc_t = Wc.rearrange("(kt p) d -> kt p d", p=P)     # (16, 128, 2048)

    with tile.TileContext(nc) as tc:
        with (
            tc.tile_pool(name="bqp", bufs=1) as bqpool,
            tc.tile_pool(name="wp", bufs=1) as wpool,
            tc.tile_pool(name="constp", bufs=1) as cpool,
            tc.tile_pool(name="qep", bufs=10) as qepool,
            tc.tile_pool(name="q2p", bufs=6) as q2pool,
            tc.tile_pool(name="o2p", bufs=8) as o2pool,
            tc.tile_pool(name="psp", bufs=8, space="PSUM") as pspool,
            tc.tile_pool(name="dramp", bufs=1, space="DRAM") as dram,
        ):
            # --- full-row W (big lines) + bq, interleaved so the k-outer
            # passes below start computing after the first pair lands ---
            bq_tiles = []
            w_tiles = []
            for k in range(KT):
                wt = wpool.tile([P, D], DT_MM, name=f"w{k}")
                nc.sync.dma_start(out=wt, in_=Wc_t[k])
                w_tiles.append(wt)
                bt = bqpool.tile([P, T], DT_MM, name=f"bq{k}")
                nc.sync.dma_start(out=bt, in_=bqT_t[k])
                bq_tiles.append(bt)
            bias_t = cpool.tile([P, D], F32, name="bias")
            nc.sync.dma_start(out=bias_t, in_=biasr[:, :])

            ain = [dram.tile([T, NW], DT_MM, name=f"ain{j}") for j in range(NQ)]
            aout = [dram.tile([T, NW], DT_MM, name=f"aout{j}") for j in range(NQ)]

            # (m, n2) chains per chunk, run k-outer in passes of <=8 banks
            chains = [(m, n2) for m in range(MT) for n2 in range(NB)]
            passes = [chains[i:i + 8] for i in range(0, len(chains), 8)]

            def stage1(j):
                with nc.named_scope(f"s1_q{j}"):
                    qe = {}
                    for m in range(MT):
                        qe[m] = qepool.tile([P, NW], DT_MM, name="qe")
                    for grp in passes:
                        ps = {c: pspool.tile([P, 512], F32, name="ps")
                              for c in grp}
                        for k in range(KT):
                            for (m, n2) in grp:
                                nc.tensor.matmul(
                                    ps[(m, n2)][:, :],
                                    bq_tiles[k][:, m * P:(m + 1) * P],
                                    w_tiles[k][:, j * NW + n2 * 512:
                                               j * NW + (n2 + 1) * 512],
                                    start=(k == 0),
                                    stop=(k == KT - 1),
                                )
                        for (m, n2) in grp:
                            nc.vector.tensor_add(
                                qe[m][:, n2 * 512:(n2 + 1) * 512],
                                ps[(m, n2)][:, :],
                                bias_t[:, j * NW + n2 * 512:
                                       j * NW + (n2 + 1) * 512],
                            )
                        # chains are m-major, so each pass of 8 covers
                        # complete m's for NB in {1,2,4} -> store them
                        for m in sorted({m for (m, _) in grp}):
                            nc.sync.dma_start(
                                out=ain[j][m * P:(m + 1) * P, :],
                                in_=qe[m][:, :],
                            )

            def a2a(j):
                nc.gpsimd.collective_compute(
                    "AllToAll",
                    mybir.AluOpType.bypass,
                    replica_groups=[list(range(NC))],
                    ins=[ain[j].opt()],
                    outs=[aout[j].opt()],
                )

            def stage2(j):
                # q2 partition packing p = r*16 + tt (r-major): the (r, t, d)
                # source view iterates to match; split across 2 queues
                ao = aout[j].rearrange("(r t) d -> r t d", r=NC)  # (8,128,NW)
                with nc.named_scope(f"s2_q{j}"):
                    for g in range(TG):
                        q2 = q2pool.tile([P, NW], DT_MM, name="q2")
                        nc.sync.dma_start(
                            out=q2[0:64, :],
                            in_=ao[0:4, g * 16:(g + 1) * 16, :],
                        )
                        nc.sync.dma_start(
                            out=q2[64:128, :],
                            in_=ao[4:8, g * 16:(g + 1) * 16, :],
                        )
                        for h in range(BG):
                            o2 = o2pool.tile([P, NW], F32, name="o2")
                            for n2 in range(NB):
                                ps2 = pspool.tile([P, 512], F32, name="ps")
                                nc.tensor.matmul(
                                    ps2[:, :],
                                    l_tiles[(g, h)][:, :],
                                    q2[:, n2 * 512:(n2 + 1) * 512],
                                    start=True,
                                    stop=True,
                                )
                                # M packing p = bb*16 + tt (b-major)
                                if (h + n2) % 2 == 0:
                                    nc.scalar.copy(
                                        o2[:, n2 * 512:(n2 + 1) * 512],
                                        ps2[:, :],
                                    )
                                else:
                                    nc.vector.tensor_copy(
                                        o2[:, n2 * 512:(n2 + 1) * 512],
                                        ps2[:, :],
                                    )
                            # out slice iterates (b, t, n) == source flat
                            # order; split by b-half across 2 queues
                            nc.sync.dma_start(
                                out=dq[j, h * 8:h * 8 + 4,
                                       g * 16:(g + 1) * 16, :],
                                in_=o2[0:64, :],
                            )
                            nc.sync.dma_start(
                                out=dq[j, h * 8 + 4:h * 8 + 8,
                                       g * 16:(g + 1) * 16, :],
                                in_=o2[64:128, :],
                            )

            # schedule: stage2(j) trails stage1 by 2 chunks
            stage1(0)
            a2a(0)
            # L tiles (stage-2 weights) load once chunk 0 is in flight
            lbig = cpool.tile([P, TG * BG * P], DT_MM, name="lbig")
            nc.sync.dma_start(out=lbig, in_=Lt[:, :])
            l_tiles = {
                (g, h): lbig[:, (g * BG + h) * P:(g * BG + h + 1) * P]
                for g in range(TG)
                for h in range(BG)
            }
            pend = []
            for j in range(1, NQ):
                stage1(j)
                a2a(j)
                pend.append(j - 1)
                if j >= 2:
                    stage2(pend.pop(0))
            for j in pend:
                stage2(j)
            stage2(NQ - 1)

    nc.finalize()
    return nc


def _prep_inputs(query_weights, basic_queries, W_mlp, b_mlp):
    qw = np.ascontiguousarray(query_weights, dtype=np.float32)
    bq = np.ascontiguousarray(basic_queries, dtype=np.float32)
    W = np.ascontiguousarray(W_mlp, dtype=np.float32)
    b = np.ascontiguousarray(b_mlp, dtype=np.float32)

    bqT = np.ascontiguousarray(bq.T.astype(NP_MM))  # (D, T), shared

    g_i = np.arange(TG)[:, None, None, None, None]
    h_i = np.arange(BG)[None, :, None, None, None]
    tt_i = np.arange(16)[None, None, :, None, None]
    r_i = np.arange(R)[None, None, None, :, None]
    bb_i = np.arange(8)[None, None, None, None, :]

    in_maps = []
    for c in range(NC):
        Wc = np.ascontiguousarray(W[:, c * D:(c + 1) * D].astype(NP_MM))
        biasr = np.ascontiguousarray(
            np.broadcast_to(b[c * D:(c + 1) * D], (P, D))
        )
        qw_c = qw[:, c * TS:(c + 1) * TS, :]  # (32, 128, 8)
        # K index r*16+tt (r-major), M index bb*16+tt (b-major)
        L = np.zeros((TG, BG, P, P), NP_MM)
        L[g_i, h_i, r_i * 16 + tt_i, bb_i * 16 + tt_i] = \
            qw_c[h_i * 8 + bb_i, g_i * 16 + tt_i, r_i].astype(NP_MM)
        # pack to (128, 32*128): Lbig[p, (g*BG+h)*128 + m] = L[g, h, p, m]
        Lbig = np.ascontiguousarray(
            L.transpose(2, 0, 1, 3).reshape(P, TG * BG * P)
        )
        in_maps.append({"bqT": bqT, "Wc": Wc, "biasr": biasr, "Lt": Lbig})
    return in_maps


last_results = None  # exposed for external profiling harnesses


def kernel(query_weights, basic_queries, W_mlp, b_mlp):
    global last_results
    if "nc" not in _cache:
        _cache["nc"] = _build_nc()
    nc = _cache["nc"]

    in_maps = _prep_inputs(query_weights, basic_queries, W_mlp, b_mlp)
    res = run_bass_kernel_spmd(nc, in_maps, core_ids=list(range(NC)))
    last_results = res

    dq = np.concatenate(
        [
            res.results[c]["dq"].transpose(1, 2, 0, 3).reshape(B, TS, D)
            for c in range(NC)
        ],
        axis=1,
    )
    basic_expanded = np.broadcast_to(
        np.ascontiguousarray(basic_queries, dtype=np.float32)[None], (B, T, D)
    )
    return dq, basic_expanded


# revision 16
# speedup vs baseline: 1.4984x; 1.4984x over previous
"""Trainium2 Bass kernel for nn_DynamicDictionaryLearning (vq_codebook).

Computation (full shapes):
    query_embed = (basic_queries @ W_mlp + b_mlp).reshape(T, R, D)    # (T, R*D)
    dynamic_queries = einsum('btr,trd->btd', query_weights, query_embed)
    basic_expanded  = broadcast(basic_queries, (B, T, D))

Sharding (8 NeuronCores, one chip):
    Stage 1 (token-MLP expansion) is tensor-sharded over the R*D output dim:
    core r computes qe_r = basic_queries @ W_mlp[:, r*D:(r+1)*D] + b_r for
    ALL tokens, reading only 1/8th of W_mlp per core.

    An on-chip AllToAll redistributes qe so core c holds all R slices for
    its 128-token slice.  Stage 2 (weighted sum over R) runs as dense PE
    matmuls with block-diagonal qw tiles: contraction packs (r x 16
    tokens) = 128 (r-major), output packs (8 batch x 16 tokens) = 128
    (b-major), so every DMA walks both sides in matching flat order with
    contiguous lines (the DMA queues are descriptor-bound).

    The pipeline is chunked over NQ D-slices; stage2(j) trails stage1 by
    two chunks so each AllToAll (~25-60us) hides under PE work.  Stage 1
    runs contraction-outermost with 8 concurrent PSUM chains so the PE
    starts as soon as the first (bq, W) tile pair lands.  Large strided
    DMAs are split in half across queues to cut their single-queue
    latency.

    basic_expanded is a pure broadcast of an input -> host-side view.
"""

import os

import numpy as np
import ml_dtypes

import concourse.bass as bass
import concourse.mybir as mybir
import concourse.tile as tile
from concourse import bacc
from concourse.bass_utils import run_bass_kernel_spmd

# Problem shapes (hardcoded per spec)
D = 2048
T = 1024
R = 8
B = 32
NC = 8
TS = T // NC          # 128 tokens per core (stage-2 ownership)
P = 128
KT = D // P           # 16 contraction tiles
MT = T // P           # 8 token tiles (stage 1)
NQ = int(os.environ.get("KNQ", "4"))   # pipeline chunks (D-slices)
NW = D // NQ          # cols per chunk
NB = NW // 512        # PSUM-bank-sized n-splits per chunk
TG = TS // 16         # 8 token groups of 16 (stage 2)
BG = B // 8           # 4 batch groups of 8 (stage 2)

F32 = mybir.dt.float32
F32R = mybir.dt.float32r
BF16 = mybir.dt.bfloat16

# matmul-operand dtype: "bf16" (fast, ~4e-3 rel err) or "f32r"
# (full fp32 data, ~3e-4 rel err, ~2x the DMA bytes)
USE_F32R = os.environ.get("KF32R", "0") == "1"
DT_MM = F32R if USE_F32R else BF16
NP_MM = np.float32 if USE_F32R else ml_dtypes.bfloat16

_cache = {}


def _build_nc():
    nc = bacc.Bacc("TRN2", target_bir_lowering=False, num_devices=NC)

    bqT = nc.dram_tensor("bqT", [D, T], DT_MM, kind="ExternalInput")
    Wc = nc.dram_tensor("Wc", [D, D], DT_MM, kind="ExternalInput")
    biasr = nc.dram_tensor("biasr", [P, D], F32, kind="ExternalInput")
    # block-diagonal qw tiles, packed (128, 32*128) for one big-line DMA
    Lt = nc.dram_tensor("Lt", [P, TG * BG * P], DT_MM, kind="ExternalInput")
    # chunk-major layout: (t, n) runs are contiguous per (j, b)
    dq = nc.dram_tensor("dq", [NQ, B, TS, NW], F32, kind="ExternalOutput")

    bqT_t = bqT.rearrange("(kt p) m -> kt p m", p=P)   # (16, 128, 1024)
    Wc_t = Wc.rearrange("(kt p) d -> kt p d", p=P)     # (16, 128, 2048)

    with tile.TileContext(nc) as tc:
        with (
            tc.tile_pool(name="bqp", bufs=1) as bqpool,
            tc.tile_pool(name="wp", bufs=1) as wpool,
            tc.tile_pool(name="constp", bufs=1) as cpool,
            tc.tile_pool(name="qep", bufs=10) as qepool,
            tc.tile_pool(name="q2p", bufs=6) as q2pool,
            tc.tile_pool(name="o2p", bufs=8) as o2pool,
            tc.tile_pool(name="psp", bufs=8, space="PSUM") as pspool,
            tc.tile_pool(name="dramp", bufs=1, space="DRAM") as dram,
        ):
            # --- full-row W (big lines) + bq, interleaved so the k-outer
            # passes below start computing after the first pair lands ---
            bq_tiles = []
            w_tiles = []
            for k in range(KT):
                wt = wpool.tile([P, D], DT_MM, name=f"w{k}")
                nc.sync.dma_start(out=wt, in_=Wc_t[k])
                w_tiles.append(wt)
                bt = bqpool.tile([P, T], DT_MM, name=f"bq{k}")
                nc.sync.dma_start(out=bt, in_=bqT_t[k])
                bq_tiles.append(bt)
            bias_t = cpool.tile([P, D], F32, name="bias")
            nc.sync.dma_start(out=bias_t, in_=biasr[:, :])

            ain = [dram.tile([T, NW], DT_MM, name=f"ain{j}") for j in range(NQ)]
            aout = [dram.tile([T, NW], DT_MM, name=f"aout{j}") for j in range(NQ)]

            # (m, n2) chains per chunk, run k-outer in passes of <=8 banks
            chains = [(m, n2) for m in range(MT) for n2 in range(NB)]
            passes = [chains[i:i + 8] for i in range(0, len(chains), 8)]

            def stage1(j):
                with nc.named_scope(f"s1_q{j}"):
                    qe = {}
                    for m in range(MT):
                        qe[m] = qepool.tile([P, NW], DT_MM, name="qe")
                    for grp in passes:
                        ps = {c: pspool.tile([P, 512], F32, name="ps")
                              for c in grp}
                        for k in range(KT):
                            for (m, n2) in grp:
                                nc.tensor.matmul(
                                    ps[(m, n2)][:, :],
                                    bq_tiles[k][:, m * P:(m + 1) * P],
                                    w_tiles[k][:, j * NW + n2 * 512:
                                               j * NW + (n2 + 1) * 512],
                                    start=(k == 0),
                                    stop=(k == KT - 1),
                                )
                        for (m, n2) in grp:
                            nc.vector.tensor_add(
                                qe[m][:, n2 * 512:(n2 + 1) * 512],
                                ps[(m, n2)][:, :],
                                bias_t[:, j * NW + n2 * 512:
                                       j * NW + (n2 + 1) * 512],
                            )
                        # chains are m-major, so each pass of 8 covers
                        # complete m's for NB in {1,2,4} -> store them
                        for m in sorted({m for (m, _) in grp}):
                            nc.sync.dma_start(
                                out=ain[j][m * P:(m + 1) * P, :],
                                in_=qe[m][:, :],
                            )

            def a2a(j):
                nc.gpsimd.collective_compute(
                    "AllToAll",
                    mybir.AluOpType.bypass,
                    replica_groups=[list(range(NC))],
                    ins=[ain[j].opt()],
                    outs=[aout[j].opt()],
                )

            def stage2(j):
                # q2 partition packing p = r*16 + tt (r-major): the (r, t, d)
                # source view iterates to match; split across 2 queues
                ao = aout[j].rearrange("(r t) d -> r t d", r=NC)  # (8,128,NW)
                with nc.named_scope(f"s2_q{j}"):
                    for g in range(TG):
                        q2 = q2pool.tile([P, NW], DT_MM, name="q2")
                        nc.sync.dma_start(
                            out=q2[:, :],
                            in_=ao[:, g * 16:(g + 1) * 16, :],
                        )
                        for h in range(BG):
                            o2 = o2pool.tile([P, NW], F32, name="o2")
                            for n2 in range(NB):
                                ps2 = pspool.tile([P, 512], F32, name="ps")
                                nc.tensor.matmul(
                                    ps2[:, :],
                                    l_tiles[(g, h)][:, :],
                                    q2[:, n2 * 512:(n2 + 1) * 512],
                                    start=True,
                                    stop=True,
                                )
                                # M packing p = bb*16 + tt (b-major)
                                if (h + n2) % 2 == 0:
                                    nc.scalar.copy(
                                        o2[:, n2 * 512:(n2 + 1) * 512],
                                        ps2[:, :],
                                    )
                                else:
                                    nc.vector.tensor_copy(
                                        o2[:, n2 * 512:(n2 + 1) * 512],
                                        ps2[:, :],
                                    )
                            # out slice iterates (b, t, n) == source
                            # flat order; issued on the ACT HWDGE ring so
                            # output traffic overlaps the SP-ring streams
                            nc.scalar.dma_start(
                                out=dq[j, h * 8:(h + 1) * 8,
                                       g * 16:(g + 1) * 16, :],
                                in_=o2[:, :],
                            )

            # schedule: stage2(j) trails stage1 by 2 chunks
            stage1(0)
            a2a(0)
            # L tiles (stage-2 weights) load once chunk 0 is in flight
            lbig = cpool.tile([P, TG * BG * P], DT_MM, name="lbig")
            nc.sync.dma_start(out=lbig, in_=Lt[:, :])
            l_tiles = {
                (g, h): lbig[:, (g * BG + h) * P:(g * BG + h + 1) * P]
                for g in range(TG)
                for h in range(BG)
            }
            pend = []
            for j in range(1, NQ):
                stage1(j)
                a2a(j)
                pend.append(j - 1)
                if j >= 2:
                    stage2(pend.pop(0))
            for j in pend:
                stage2(j)
            stage2(NQ - 1)

    nc.finalize()
    return nc


def _prep_inputs(query_weights, basic_queries, W_mlp, b_mlp):
    qw = np.ascontiguousarray(query_weights, dtype=np.float32)
    bq = np.ascontiguousarray(basic_queries, dtype=np.float32)
    W = np.ascontiguousarray(W_mlp, dtype=np.float32)
    b = np.ascontiguousarray(b_mlp, dtype=np.float32)

    bqT = np.ascontiguousarray(bq.T.astype(NP_MM))  # (D, T), shared

    g_i = np.arange(TG)[:, None, None, None, None]
    h_i = np.arange(BG)[None, :, None, None, None]
    tt_i = np.arange(16)[None, None, :, None, None]
    r_i = np.arange(R)[None, None, None, :, None]
    bb_i = np.arange(8)[None, None, None, None, :]

    in_maps = []
    for c in range(NC):
        Wc = np.ascontiguousarray(W[:, c * D:(c + 1) * D].astype(NP_MM))
        biasr = np.ascontiguousarray(
            np.broadcast_to(b[c * D:(c + 1) * D], (P, D))
        )
        qw_c = qw[:, c * TS:(c + 1) * TS, :]  # (32, 128, 8)
        # K index r*16+tt (r-major), M index bb*16+tt (b-major)
        L = np.zeros((TG, BG, P, P), NP_MM)
        L[g_i, h_i, r_i * 16 + tt_i, bb_i * 16 + tt_i] = \
            qw_c[h_i * 8 + bb_i, g_i * 16 + tt_i, r_i].astype(NP_MM)
        # pack to (128, 32*128): Lbig[p, (g*BG+h)*128 + m] = L[g, h, p, m]
        Lbig = np.ascontiguousarray(
            L.transpose(2, 0, 1, 3).reshape(P, TG * BG * P)
        )
        in_maps.append({"bqT": bqT, "Wc": Wc, "biasr": biasr, "Lt": Lbig})
    return in_maps


last_results = None  # exposed for external profiling harnesses


def kernel(query_weights, basic_queries, W_mlp, b_mlp):
    global last_results
    if "nc" not in _cache:
        _cache["nc"] = _build_nc()
    nc = _cache["nc"]

    in_maps = _prep_inputs(query_weights, basic_queries, W_mlp, b_mlp)
    res = run_bass_kernel_spmd(nc, in_maps, core_ids=list(range(NC)))
    last_results = res

    dq = np.concatenate(
        [
            res.results[c]["dq"].transpose(1, 2, 0, 3).reshape(B, TS, D)
            for c in range(NC)
        ],
        axis=1,
    )
    basic_expanded = np.broadcast_to(
        np.ascontiguousarray(basic_queries, dtype=np.float32)[None], (B, T, D)
    )
    return dq, basic_expanded


# revision 17
# speedup vs baseline: 1.5160x; 1.0118x over previous
"""Trainium2 Bass kernel for nn_DynamicDictionaryLearning (vq_codebook).

Computation (full shapes):
    query_embed = (basic_queries @ W_mlp + b_mlp).reshape(T, R, D)    # (T, R*D)
    dynamic_queries = einsum('btr,trd->btd', query_weights, query_embed)
    basic_expanded  = broadcast(basic_queries, (B, T, D))

Sharding (8 NeuronCores, one chip):
    Stage 1 (token-MLP expansion) is tensor-sharded over the R*D output dim:
    core r computes qe_r = basic_queries @ W_mlp[:, r*D:(r+1)*D] + b_r for
    ALL tokens, reading only 1/8th of W_mlp per core.

    An on-chip AllToAll redistributes qe so core c holds all R slices for
    its 128-token slice.  Stage 2 (weighted sum over R) runs as dense PE
    matmuls with block-diagonal qw tiles: contraction packs (r x 16
    tokens) = 128 (r-major), output packs (8 batch x 16 tokens) = 128
    (b-major), so every DMA walks both sides in matching flat order with
    contiguous lines (the DMA queues are descriptor-bound).

    The pipeline is chunked over NQ D-slices; stage2(j) trails stage1 by
    two chunks so each AllToAll (~25-60us) hides under PE work.  Stage 1
    runs contraction-outermost with 8 concurrent PSUM chains so the PE
    starts as soon as the first (bq, W) tile pair lands.  Large strided
    Stage-1 k-outer passes hold 4 PSUM banks so stage-2 matmuls and the
    next pass always have banks available.

    basic_expanded is a pure broadcast of an input -> host-side view.
"""

import os

import numpy as np
import ml_dtypes

import concourse.bass as bass
import concourse.mybir as mybir
import concourse.tile as tile
from concourse import bacc
from concourse.bass_utils import run_bass_kernel_spmd

# Problem shapes (hardcoded per spec)
D = 2048
T = 1024
R = 8
B = 32
NC = 8
TS = T // NC          # 128 tokens per core (stage-2 ownership)
P = 128
KT = D // P           # 16 contraction tiles
MT = T // P           # 8 token tiles (stage 1)
NQ = int(os.environ.get("KNQ", "4"))   # pipeline chunks (D-slices)
NW = D // NQ          # cols per chunk
NB = NW // 512        # PSUM-bank-sized n-splits per chunk
TG = TS // 16         # 8 token groups of 16 (stage 2)
BG = B // 8           # 4 batch groups of 8 (stage 2)

F32 = mybir.dt.float32
F32R = mybir.dt.float32r
BF16 = mybir.dt.bfloat16

# matmul-operand dtype: "bf16" (fast, ~4e-3 rel err) or "f32r"
# (full fp32 data, ~3e-4 rel err, ~2x the DMA bytes)
USE_F32R = os.environ.get("KF32R", "0") == "1"
DT_MM = F32R if USE_F32R else BF16
NP_MM = np.float32 if USE_F32R else ml_dtypes.bfloat16

_cache = {}


def _build_nc():
    nc = bacc.Bacc("TRN2", target_bir_lowering=False, num_devices=NC)

    bqT = nc.dram_tensor("bqT", [D, T], DT_MM, kind="ExternalInput")
    Wc = nc.dram_tensor("Wc", [D, D], DT_MM, kind="ExternalInput")
    biasr = nc.dram_tensor("biasr", [P, D], F32, kind="ExternalInput")
    # block-diagonal qw tiles, packed (128, 32*128) for one big-line DMA
    Lt = nc.dram_tensor("Lt", [P, TG * BG * P], DT_MM, kind="ExternalInput")
    # chunk-major layout: (t, n) runs are contiguous per (j, b)
    dq = nc.dram_tensor("dq", [NQ, B, TS, NW], F32, kind="ExternalOutput")

    bqT_t = bqT.rearrange("(kt p) m -> kt p m", p=P)   # (16, 128, 1024)
    Wc_t = Wc.rearrange("(kt p) d -> kt p d", p=P)     # (16, 128, 2048)

    with tile.TileContext(nc) as tc:
        with (
            tc.tile_pool(name="bqp", bufs=1) as bqpool,
            tc.tile_pool(name="wp", bufs=1) as wpool,
            tc.tile_pool(name="constp", bufs=1) as cpool,
            tc.tile_pool(name="qep", bufs=10) as qepool,
            tc.tile_pool(name="q2p", bufs=6) as q2pool,
            tc.tile_pool(name="o2p", bufs=8) as o2pool,
            tc.tile_pool(name="psp", bufs=8, space="PSUM") as pspool,
            tc.tile_pool(name="dramp", bufs=1, space="DRAM") as dram,
        ):
            # --- full-row W (big lines) + bq, interleaved so the k-outer
            # passes below start computing after the first pair lands ---
            bq_tiles = []
            w_tiles = []
            for k in range(KT):
                wt = wpool.tile([P, D], DT_MM, name=f"w{k}")
                nc.sync.dma_start(out=wt, in_=Wc_t[k])
                w_tiles.append(wt)
                bt = bqpool.tile([P, T], DT_MM, name=f"bq{k}")
                nc.sync.dma_start(out=bt, in_=bqT_t[k])
                bq_tiles.append(bt)
            bias_t = cpool.tile([P, D], F32, name="bias")
            nc.sync.dma_start(out=bias_t, in_=biasr[:, :])

            ain = [dram.tile([T, NW], DT_MM, name=f"ain{j}") for j in range(NQ)]
            aout = [dram.tile([T, NW], DT_MM, name=f"aout{j}") for j in range(NQ)]

            # (m, n2) chains per chunk, run k-outer in passes of 4 banks
            # (leaves 4 PSUM banks rolling for stage2 / the next pass)
            chains = [(m, n2) for m in range(MT) for n2 in range(NB)]
            passes = [chains[i:i + 4] for i in range(0, len(chains), 4)]

            def stage1(j):
                with nc.named_scope(f"s1_q{j}"):
                    qe = {}
                    for m in range(MT):
                        qe[m] = qepool.tile([P, NW], DT_MM, name="qe")
                    for grp in passes:
                        ps = {c: pspool.tile([P, 512], F32, name="ps")
                              for c in grp}
                        for k in range(KT):
                            for (m, n2) in grp:
                                nc.tensor.matmul(
                                    ps[(m, n2)][:, :],
                                    bq_tiles[k][:, m * P:(m + 1) * P],
                                    w_tiles[k][:, j * NW + n2 * 512:
                                               j * NW + (n2 + 1) * 512],
                                    start=(k == 0),
                                    stop=(k == KT - 1),
                                )
                        for (m, n2) in grp:
                            nc.vector.tensor_add(
                                qe[m][:, n2 * 512:(n2 + 1) * 512],
                                ps[(m, n2)][:, :],
                                bias_t[:, j * NW + n2 * 512:
                                       j * NW + (n2 + 1) * 512],
                            )
                        # chains are m-major, so each pass of 8 covers
                        # complete m's for NB in {1,2,4} -> store them
                        for m in sorted({m for (m, _) in grp}):
                            nc.sync.dma_start(
                                out=ain[j][m * P:(m + 1) * P, :],
                                in_=qe[m][:, :],
                            )

            def a2a(j):
                nc.gpsimd.collective_compute(
                    "AllToAll",
                    mybir.AluOpType.bypass,
                    replica_groups=[list(range(NC))],
                    ins=[ain[j].opt()],
                    outs=[aout[j].opt()],
                )

            def stage2(j):
                # q2 partition packing p = r*16 + tt (r-major): the (r, t, d)
                # source view iterates to match; split across 2 queues
                ao = aout[j].rearrange("(r t) d -> r t d", r=NC)  # (8,128,NW)
                with nc.named_scope(f"s2_q{j}"):
                    for g in range(TG):
                        q2 = q2pool.tile([P, NW], DT_MM, name="q2")
                        nc.sync.dma_start(
                            out=q2[:, :],
                            in_=ao[:, g * 16:(g + 1) * 16, :],
                        )
                        for h in range(BG):
                            o2 = o2pool.tile([P, NW], F32, name="o2")
                            for n2 in range(NB):
                                ps2 = pspool.tile([P, 512], F32, name="ps")
                                nc.tensor.matmul(
                                    ps2[:, :],
                                    l_tiles[(g, h)][:, :],
                                    q2[:, n2 * 512:(n2 + 1) * 512],
                                    start=True,
                                    stop=True,
                                )
                                # M packing p = bb*16 + tt (b-major)
                                if (h + n2) % 2 == 0:
                                    nc.scalar.copy(
                                        o2[:, n2 * 512:(n2 + 1) * 512],
                                        ps2[:, :],
                                    )
                                else:
                                    nc.vector.tensor_copy(
                                        o2[:, n2 * 512:(n2 + 1) * 512],
                                        ps2[:, :],
                                    )
                            # out slice iterates (b, t, n) == source
                            # flat order; issued on the ACT HWDGE ring so
                            # output traffic overlaps the SP-ring streams
                            nc.scalar.dma_start(
                                out=dq[j, h * 8:(h + 1) * 8,
                                       g * 16:(g + 1) * 16, :],
                                in_=o2[:, :],
                            )

            # schedule: stage2(j) trails stage1 by 2 chunks
            stage1(0)
            a2a(0)
            # L tiles (stage-2 weights) load once chunk 0 is in flight
            lbig = cpool.tile([P, TG * BG * P], DT_MM, name="lbig")
            nc.sync.dma_start(out=lbig, in_=Lt[:, :])
            l_tiles = {
                (g, h): lbig[:, (g * BG + h) * P:(g * BG + h + 1) * P]
                for g in range(TG)
                for h in range(BG)
            }
            pend = []
            for j in range(1, NQ):
                stage1(j)
                a2a(j)
                pend.append(j - 1)
                if j >= 2:
                    stage2(pend.pop(0))
            for j in pend:
                stage2(j)
            stage2(NQ - 1)

    nc.finalize()
    return nc


def _prep_inputs(query_weights, basic_queries, W_mlp, b_mlp):
    qw = np.ascontiguousarray(query_weights, dtype=np.float32)
    bq = np.ascontiguousarray(basic_queries, dtype=np.float32)
    W = np.ascontiguousarray(W_mlp, dtype=np.float32)
    b = np.ascontiguousarray(b_mlp, dtype=np.float32)

    bqT = np.ascontiguousarray(bq.T.astype(NP_MM))  # (D, T), shared

    g_i = np.arange(TG)[:, None, None, None, None]
    h_i = np.arange(BG)[None, :, None, None, None]
    tt_i = np.arange(16)[None, None, :, None, None]
    r_i = np.arange(R)[None, None, None, :, None]
    bb_i = np.arange(8)[None, None, None, None, :]

    in_maps = []
    for c in range(NC):
        Wc = np.ascontiguousarray(W[:, c * D:(c + 1) * D].astype(NP_MM))
        biasr = np.ascontiguousarray(
            np.broadcast_to(b[c * D:(c + 1) * D], (P, D))
        )
        qw_c = qw[:, c * TS:(c + 1) * TS, :]  # (32, 128, 8)
        # K index r*16+tt (r-major), M index bb*16+tt (b-major)
        L = np.zeros((TG, BG, P, P), NP_MM)
        L[g_i, h_i, r_i * 16 + tt_i, bb_i * 16 + tt_i] = \
            qw_c[h_i * 8 + bb_i, g_i * 16 + tt_i, r_i].astype(NP_MM)
        # pack to (128, 32*128): Lbig[p, (g*BG+h)*128 + m] = L[g, h, p, m]
        Lbig = np.ascontiguousarray(
            L.transpose(2, 0, 1, 3).reshape(P, TG * BG * P)
        )
        in_maps.append({"bqT": bqT, "Wc": Wc, "biasr": biasr, "Lt": Lbig})
    return in_maps


last_results = None  # exposed for external profiling harnesses


def kernel(query_weights, basic_queries, W_mlp, b_mlp):
    global last_results
    if "nc" not in _cache:
        _cache["nc"] = _build_nc()
    nc = _cache["nc"]

    in_maps = _prep_inputs(query_weights, basic_queries, W_mlp, b_mlp)
    res = run_bass_kernel_spmd(nc, in_maps, core_ids=list(range(NC)))
    last_results = res

    dq = np.concatenate(
        [
            res.results[c]["dq"].transpose(1, 2, 0, 3).reshape(B, TS, D)
            for c in range(NC)
        ],
        axis=1,
    )
    basic_expanded = np.broadcast_to(
        np.ascontiguousarray(basic_queries, dtype=np.float32)[None], (B, T, D)
    )
    return dq, basic_expanded
